# revision 1
# baseline (speedup 1.0000x reference)
"""Bilinear grid sample on 8 Trainium2 NeuronCores.

Data-parallel over batch: each core handles 2 of the 16 batches.
Per batch: points are processed 128 at a time (one per SBUF partition).
A single indirect DMA gathers, per point, the two needed image rows
(h_floor and h_floor+1), each as 2 adjacent w-columns x 256 channels
(512 contiguous floats = 2KiB per descriptor).  Fused DVE ops then do
the bilinear interpolation with per-partition scalar weights, writing
into [128, 16*256] chunks that are stored with large contiguous DMAs.
"""

import numpy as np

import concourse.bass as bass
import concourse.mybir as mybir
import concourse.tile as tile
from concourse.bass_utils import run_bass_kernel_spmd

B, H, W, C, P = 16, 128, 128, 256, 8192
NCORES = 8
BPC = B // NCORES        # batches per core
PTILE = 128              # points per gather tile (one per partition)
TPB = P // PTILE         # gather tiles per batch
OCHUNK = 16              # gather tiles per output store

_f32 = mybir.dt.float32
_i32 = mybir.dt.int32
_mul = mybir.AluOpType.mult
_add = mybir.AluOpType.add
_sub = mybir.AluOpType.subtract
_mod = mybir.AluOpType.mod


def build_nc() -> bass.Bass:
    nc = bass.Bass("TRN2")
    x = nc.dram_tensor("x", [BPC * H * W, C], _f32, kind="ExternalInput")
    idx = nc.dram_tensor("idx", [BPC * P, 2], _f32, kind="ExternalInput")
    out = nc.dram_tensor("out", [BPC * P, C], _f32, kind="ExternalOutput")

    kpp = P // PTILE     # points per partition per batch (64)

    with tile.TileContext(nc) as tc:
        with (
            tc.tile_pool(name="ip", bufs=2) as ip,
            tc.tile_pool(name="gp", bufs=4) as gp,
            tc.tile_pool(name="wp", bufs=3) as wp,
            tc.tile_pool(name="op", bufs=2) as op,
        ):
            for lb in range(BPC):
                # --- index prep: [128, 2*kpp] raw (h,w) pairs, point
                # (partition p, slot t) = global point p*kpp + t
                raw = ip.tile([PTILE, 2 * kpp], _f32, tag="raw")
                nc.sync.dma_start(
                    raw[:],
                    idx[lb * P:(lb + 1) * P, :].rearrange(
                        "(p k) c -> p (k c)", p=PTILE
                    ),
                )
                # floor via the round-to-nearest magic constant: rnd = RN(x),
                # flr = rnd - (rnd > x); exact in f32 for x in [0, 2^23).
                rnd = ip.tile([PTILE, 2 * kpp], _f32, tag="rnd")
                nc.vector.tensor_scalar(
                    rnd[:], raw[:], 8388608.0, 8388608.0, _add, _sub
                )
                gt = ip.tile([PTILE, 2 * kpp], _f32, tag="gt")
                nc.vector.tensor_tensor(gt[:], rnd[:], raw[:],
                                        mybir.AluOpType.is_gt)
                flr = ip.tile([PTILE, 2 * kpp], _f32, tag="flr")
                nc.vector.tensor_tensor(flr[:], rnd[:], gt[:], _sub)
                mu = ip.tile([PTILE, 2 * kpp], _f32, tag="mu")
                nc.vector.tensor_tensor(mu[:], raw[:], flr[:], _sub)
                # top row id (fp32, exact): hf*W + wf  (+ lb*H*W batch base)
                topf = ip.tile([PTILE, kpp], _f32, tag="topf")
                nc.vector.scalar_tensor_tensor(
                    topf[:], flr[:, 0::2], float(W), flr[:, 1::2], _mul, _add
                )
                ids = ip.tile([PTILE, 2 * kpp], _i32, tag="ids")
                nc.vector.tensor_scalar(
                    ids[:, 0::2], topf[:], float(lb * H * W), None, _add
                )
                nc.vector.tensor_scalar(
                    ids[:, 1::2], topf[:], float(lb * H * W + W), None, _add
                )

                # --- per 128-point tile: gather + interpolate
                for t in range(TPB):
                    # HW indirect DMA semantics: one index per partition,
                    # filling that partition's whole dest row contiguously.
                    gt_ = gp.tile([PTILE, 2 * C], _f32, tag="gt")
                    nc.gpsimd.indirect_dma_start(
                        out=gt_[:],
                        out_offset=None,
                        in_=x[:],
                        in_offset=bass.IndirectOffsetOnAxis(
                            ap=ids[:, 2 * t:2 * t + 1], axis=0
                        ),
                    )
                    gb = gp.tile([PTILE, 2 * C], _f32, tag="gb")
                    nc.gpsimd.indirect_dma_start(
                        out=gb[:],
                        out_offset=None,
                        in_=x[:],
                        in_offset=bass.IndirectOffsetOnAxis(
                            ap=ids[:, 2 * t + 1:2 * t + 2], axis=0
                        ),
                    )
                    mx = mu[:, 2 * t:2 * t + 1]
                    my = mu[:, 2 * t + 1:2 * t + 2]
                    # Touch gt_ on DVE so its completion sem is observed
                    # before d, which then only needs to wait on gb
                    # (single wait slot per instruction).
                    tch = wp.tile([PTILE, 1], _f32, tag="tch")
                    nc.vector.tensor_copy(tch[:], gt_[:, :1])
                    # d = bottom - top
                    d = wp.tile([PTILE, 2 * C], _f32, tag="d")
                    nc.vector.tensor_tensor(d[:], gb[:], gt_[:], _sub)
                    # r = d*mx + top     (h-interpolated [left|right])
                    r = wp.tile([PTILE, 2 * C], _f32, tag="r")
                    nc.vector.scalar_tensor_tensor(
                        r[:], d[:], mx, gt_[:], _mul, _add
                    )
                    # e = right - left
                    e = wp.tile([PTILE, C], _f32, tag="e")
                    nc.vector.tensor_tensor(e[:], r[:, C:], r[:, :C], _sub)
                    # out = e*my + left
                    if t % OCHUNK == 0:
                        och = op.tile([PTILE, OCHUNK * C], _f32, tag="och")
                    j = t % OCHUNK
                    nc.vector.scalar_tensor_tensor(
                        och[:, j * C:(j + 1) * C], e[:], my, r[:, :C], _mul, _add
                    )
                    if j == OCHUNK - 1:
                        c0 = (t // OCHUNK) * OCHUNK
                        dst = out[lb * P:(lb + 1) * P, :].rearrange(
                            "(p k) c -> p (k c)", p=PTILE
                        )[:, c0 * C:(c0 + OCHUNK) * C]
                        nc.sync.dma_start(dst, och[:])
                        # Touch the chunk on DVE after the store so the DVE
                        # proc observes the store's completion sem: the next
                        # writer of this slot then needs no extra wait, and
                        # the tail drain's DMA waits become redundant.
                        nc.vector.memset(och[:, :1], 0.0)

    _legalize_waits(nc)
    return nc


def _legalize_waits(nc: bass.Bass) -> None:
    """Every instruction has a single sync-wait slot in this walrus codegen.
    Tile emits per-proc minimal waits but is not transitively minimal, so
    DMA-completion waits show up alongside an engine wait that already
    implies them:
      - gather DMA slot reuse: the DVE readers of the previous gather waited
        on its completion sem, so the WAR wait on those readers implies it;
      - the kernel-tail drain: every DMA completion sem is observed by DVE
        (gathers via their readers, stores via the post-store memsets), so
        the DVE-retirement wait implies all of them.
    Keep only the engine wait in those cases."""
    for bb in nc.m.functions[0].blocks:
        for ins in bb.instructions:
            si = ins.sync_info
            if si is None or len(si.on_wait) <= 1:
                continue
            kind = type(ins).__name__
            assert kind in ("InstDMACopy", "InstDrain"), (ins.name, kind)
            keep = [w for w in si.on_wait if not w.ant_name.startswith("DMA")]
            drop = [w for w in si.on_wait if w.ant_name.startswith("DMA")]
            assert len(keep) == 1 and keep[0].ant_name.startswith("DVE"), (
                ins.name, si.on_wait)
            assert all(w.ant_name.startswith("DMASW") or
                       w.ant_name.startswith("DMAHW") for w in drop), si.on_wait
            si.on_wait = keep


_NC = None


def _get_nc() -> bass.Bass:
    global _NC
    if _NC is None:
        _NC = build_nc()
    return _NC


def kernel(in_tensor: np.ndarray, indices: np.ndarray) -> np.ndarray:
    nc = _get_nc()
    in_maps = []
    for i in range(NCORES):
        in_maps.append(
            {
                "x": np.ascontiguousarray(
                    in_tensor[i * BPC:(i + 1) * BPC], dtype=np.float32
                ).reshape(BPC * H * W, C),
                "idx": np.ascontiguousarray(
                    indices[i * BPC:(i + 1) * BPC], dtype=np.float32
                ).reshape(BPC * P, 2),
            }
        )
    res = run_bass_kernel_spmd(nc, in_maps, core_ids=list(range(NCORES)))
    return np.concatenate(
        [res.results[i]["out"].reshape(BPC, P, C) for i in range(NCORES)], axis=0
    )

